# revision 1
# baseline (speedup 1.0000x reference)
"""Trainium2 Bass kernel for LoRA attention prefill (B=4, S=1024, D=4096, H=32).

Sharding: tensor-parallel over heads. Each of the 8 cores computes 4 heads
(512 of the 4096 q/k/v features, column-shard of wq/wk/wv + LoRA B) and a
row-shard of wo, producing a full-shape [T, D] partial output; partials are
summed on the host.

Device layout choices:
  - All matmuls in bf16 with fp32 PSUM accumulation.
  - Activations kept feature-on-partition ("transposed"): xT [D, T] feeds
    Q/K projections directly as PE operands; V is computed token-on-partition
    so it can serve as the PV-matmul stationary operand without transposes.
  - RoPE handled by host-permuting wq/wk rows so each head's real/imag halves
    land in lane-aligned 64-partition blocks of separate psum tiles; scores
    then contract each head with two K=64 matmuls.
  - Attention in "scoresT" layout (keys on partitions): softmax denominator
    via a ones-column matmul on PE, probs feed the PV matmul untransposed,
    normalization applied to the attention output via a PE broadcast of the
    reciprocal sums.
"""
import sys
from contextlib import ExitStack

sys.path.insert(0, "/opt/trn_rl_repo")

import numpy as np
import ml_dtypes

import concourse.bass as bass
import concourse.mybir as mybir
import concourse.tile as tile
from concourse import bacc
from concourse.bass_utils import run_bass_kernel_spmd
from concourse.tile import TileContext

B, S, D = 4, 1024, 4096
H, HD = 32, 128
R = 16
LORA_SCALE = 2.0
N_CORES = 8
HPC = H // N_CORES            # heads per core
FPC = HPC * HD                # features per core = 512
T = B * S                     # 4096 tokens
TT = 256                      # stage-A T-tile (tokens)
NTT = S // TT                 # T-tiles per batch = 4
SCALE = float(1.0 / np.sqrt(HD))
BF = mybir.dt.bfloat16
F32 = mybir.dt.float32


def _bf(a):
    return np.ascontiguousarray(np.asarray(a, np.float32).astype(ml_dtypes.bfloat16))


def _core_perm(c):
    hs = [HPC * c + i for i in range(HPC)]
    ev = np.arange(0, HD, 2)
    od = np.arange(1, HD, 2)
    out = []
    for pair in (0, 1):
        h0, h1 = hs[2 * pair], hs[2 * pair + 1]
        out.append(h0 * HD + ev)
        out.append(h1 * HD + ev)
        out.append(h0 * HD + od)
        out.append(h1 * HD + od)
    return np.concatenate(out)


def _check_causal(mask):
    iu = np.triu_indices(S, k=1)
    il = np.tril_indices(S, k=0)
    return bool((mask[iu] <= -1e8).all() and (mask[il] == 0).all())


def _host_prep(x, wq_w, wq_a, wq_b, wk_w, wv_w, wv_a, wv_b, wo_w,
               freqs_cos, freqs_sin, mask):
    x2 = np.asarray(x, np.float32).reshape(T, D)
    xT = _bf(x2.T)
    waT = np.zeros((D, 48), np.float32)
    waT[:, 0:16] = np.asarray(wq_a, np.float32).T
    waT[:, 32:48] = np.asarray(wv_a, np.float32).T
    waT = _bf(waT)

    cosT = np.asarray(freqs_cos, np.float32).T
    sinT = np.asarray(freqs_sin, np.float32).T
    cc = np.ascontiguousarray(np.tile(cosT, (2, B)).astype(np.float32))
    ss = np.ascontiguousarray(np.tile(sinT, (2, B)).astype(np.float32))

    mask = np.asarray(mask, np.float32)
    causal = _check_causal(mask)
    mT = mask.T * np.float32(np.sqrt(HD))
    if causal:
        # diag-band blocks are identical for both query halves; verify
        maskp = np.zeros((4, 128, 512), np.float32)
        for j in range(4):
            maskp[j] = mT[j * 128:(j + 1) * 128, 0:512]
            if not np.array_equal(
                    maskp[j], mT[(4 + j) * 128:(5 + j) * 128, 512:1024]):
                causal = False
                break
    if not causal:
        maskp = np.zeros((8, 128, 2, 512), np.float32)
        for qh in range(2):
            for j in range(8):
                maskp[j, :, qh, :] = mT[j * 128:(j + 1) * 128,
                                        qh * 512:(qh + 1) * 512]

    shared = dict(xT=xT, waT=waT, cc=cc, ss=ss, maskp=maskp)
    cores = []
    for c in range(N_CORES):
        perm = _core_perm(c)
        sl = slice(c * FPC, (c + 1) * FPC)
        cores.append(dict(
            wqT=_bf(np.asarray(wq_w, np.float32)[perm, :].T),
            wkT=_bf(np.asarray(wk_w, np.float32)[perm, :].T),
            wvT=_bf(np.asarray(wv_w, np.float32)[sl, :].T),
            wqbT=_bf((np.asarray(wq_b, np.float32)[perm, :] * LORA_SCALE).T),
            wvbT=_bf((np.asarray(wv_b, np.float32)[sl, :] * LORA_SCALE).T),
            woT=_bf(np.asarray(wo_w, np.float32)[:, sl].T),
        ))
    return shared, cores, causal


def _build_program(causal):
    nc = bacc.Bacc("TRN2", num_devices=N_CORES)
    dt = mybir.dt
    nkb = 4 if causal else 8

    xT = nc.dram_tensor("xT", [D, T], BF, kind="ExternalInput").ap()
    wqT = nc.dram_tensor("wqT", [D, FPC], BF, kind="ExternalInput").ap()
    wkT = nc.dram_tensor("wkT", [D, FPC], BF, kind="ExternalInput").ap()
    wvT = nc.dram_tensor("wvT", [D, FPC], BF, kind="ExternalInput").ap()
    waT = nc.dram_tensor("waT", [D, 48], BF, kind="ExternalInput").ap()
    wqbT = nc.dram_tensor("wqbT", [R, FPC], BF, kind="ExternalInput").ap()
    wvbT = nc.dram_tensor("wvbT", [R, FPC], BF, kind="ExternalInput").ap()
    woT = nc.dram_tensor("woT", [FPC, D], BF, kind="ExternalInput").ap()
    cc = nc.dram_tensor("cc", [128, T], F32, kind="ExternalInput").ap()
    ss = nc.dram_tensor("ss", [128, T], F32, kind="ExternalInput").ap()
    mshape = [4, 128, 512] if causal else [8, 128, 2, 512]
    maskp = nc.dram_tensor("maskp", mshape, F32, kind="ExternalInput").ap()
    y = nc.dram_tensor("y", [T, D], F32, kind="ExternalOutput").ap()

    with TileContext(nc) as tc, ExitStack() as ctx:
        wpool = ctx.enter_context(tc.tile_pool(name="wpool", bufs=1))
        xpool = ctx.enter_context(tc.tile_pool(name="xpool", bufs=2))
        ccp = ctx.enter_context(tc.tile_pool(name="ccp", bufs=1))
        qkvp = ctx.enter_context(tc.tile_pool(name="qkvp", bufs=1))
        xap = ctx.enter_context(tc.tile_pool(name="xap", bufs=2))
        expp = ctx.enter_context(tc.tile_pool(name="expp", bufs=3))
        otp = ctx.enter_context(tc.tile_pool(name="otp", bufs=1))
        outp = ctx.enter_context(tc.tile_pool(name="outp", bufs=3))
        tmpp = ctx.enter_context(tc.tile_pool(name="tmpp", bufs=6))
        stp = ctx.enter_context(tc.tile_pool(name="stp", bufs=4))
        sump = ctx.enter_context(tc.tile_pool(name="sump", bufs=1))
        wop = ctx.enter_context(tc.tile_pool(name="wop", bufs=2))
        psa = ctx.enter_context(tc.tile_pool(name="psac", bufs=3, space="PSUM"))
        psc = psa
        psb = ctx.enter_context(tc.tile_pool(name="psb", bufs=5, space="PSUM"))

        # resident weights
        wq_sb = wpool.tile([128, 32, FPC], BF, tag="wq")
        nc.sync.dma_start(wq_sb[:], wqT.rearrange("(o p) f -> p o f", p=128))
        wk_sb = wpool.tile([128, 32, FPC], BF, tag="wk")
        nc.sync.dma_start(wk_sb[:], wkT.rearrange("(o p) f -> p o f", p=128))
        wv_sb = wpool.tile([128, 32, FPC], BF, tag="wv")
        nc.sync.dma_start(wv_sb[:], wvT.rearrange("(o p) f -> p o f", p=128))
        wa_sb = wpool.tile([128, 32, 48], BF, tag="wa")
        nc.sync.dma_start(wa_sb[:], waT.rearrange("(o p) f -> p o f", p=128))
        wqb_sb = wpool.tile([R, FPC], BF, tag="wqb")
        nc.sync.dma_start(wqb_sb[:], wqbT[:])
        # parked at partitions 32-47 so the V-lora matmul's lhsT (xa rows
        # 32-47) and rhs share a base partition, as the PE requires
        wvb_sb = wpool.tile([48, FPC], BF, tag="wvb")
        nc.sync.dma_start(wvb_sb[32:48, :], wvbT[:])
        if causal:
            mask_sb = wpool.tile([128, 4, 512], F32, tag="mask")
            nc.sync.dma_start(mask_sb[:], maskp.rearrange("j p n -> p j n"))
        else:
            mask_sb = wpool.tile([128, 8, 2, 512], F32, tag="mask")
            nc.sync.dma_start(mask_sb[:],
                              maskp.rearrange("j p q n -> p j q n"))
        ones_col = wpool.tile([128, 1], BF, tag="onec")
        nc.gpsimd.memset(ones_col[:], 1.0)
        ones_row = wpool.tile([1, 128], F32, tag="oner")
        nc.gpsimd.memset(ones_row[:], 1.0)

        for b in range(B):
            Q_sb = qkvp.tile([128, 4, S], BF, tag="Qsb")
            K_sb = qkvp.tile([128, 4, S], BF, tag="Ksb")
            V_sb = qkvp.tile([128, 8, FPC], BF, tag="Vsb")
            OT_sb = otp.tile([128, 4, S], BF, tag="OT")

            def attn_half(qh):
                q0 = qh * 512
                kbs = list(range(0, qh * 4 + 4)) if causal else list(range(8))
                last = len(kbs) - 1
                for l in range(HPC):
                    ps_ot = psb.tile([128, 512], F32, tag="psb")
                    ps_sum = psb.tile([128, 512], F32, tag="psb")
                    for j, kb in enumerate(kbs):
                        k0 = kb * 128
                        ps_sc = psb.tile([128, 512], F32, tag="psb")
                        nc.tensor.matmul(
                            ps_sc[:], K_sb[:, l, k0:k0 + 128],
                            Q_sb[:, l, q0:q0 + 512], start=True, stop=True)
                        if causal:
                            if kb >= qh * 4:
                                nc.vector.tensor_add(
                                    ps_sc[:], ps_sc[:],
                                    mask_sb[:, kb - qh * 4, :])
                        else:
                            nc.vector.tensor_add(
                                ps_sc[:], ps_sc[:], mask_sb[:, kb, qh, :])
                        e_sb = expp.tile([128, 512], BF, tag="e")
                        nc.scalar.activation(
                            e_sb[:], ps_sc[:],
                            mybir.ActivationFunctionType.Exp, scale=SCALE)
                        nc.tensor.matmul(ps_sum[0:1, :], ones_col[:], e_sb[:],
                                         start=(j == 0), stop=(j == last))
                        nc.tensor.matmul(
                            ps_ot[:], V_sb[:, kb, l * 128:(l + 1) * 128],
                            e_sb[:], start=(j == 0), stop=(j == last))
                    # normalization: keep the slow reciprocal off PSUM so the
                    # next head's matmuls aren't starved of banks
                    sum_sb = sump.tile([1, 512], F32, tag="sum")
                    nc.scalar.copy(sum_sb[:], ps_sum[0:1, :])
                    rec1_sb = sump.tile([1, 512], F32, tag="rec1")
                    nc.vector.reciprocal(rec1_sb[:], sum_sb[:])
                    ps_bc = psb.tile([128, 512], F32, tag="psb")
                    nc.tensor.matmul(ps_bc[:], ones_row[:], rec1_sb[:],
                                     start=True, stop=True)
                    rec_sb = outp.tile([128, 512], F32, tag="o")
                    nc.vector.tensor_copy(rec_sb[:], ps_bc[:])
                    nc.vector.tensor_mul(OT_sb[:, l, q0:q0 + 512], ps_ot[:],
                                         rec_sb[:])

            # ---- stage A: projections + RoPE (attention qh=0 interleaved) --
            for tt in range(NTT):
                t0 = b * S + tt * TT
                x_sb = xpool.tile([128, 32, TT], BF, tag="x")
                nc.sync.dma_start(
                    x_sb[:],
                    xT.rearrange("(o p) t -> p o t", p=128)[:, :, t0:t0 + TT])
                cc_sb = ccp.tile([128, TT], F32, tag="cc")
                nc.sync.dma_start(cc_sb[:], cc[:, t0:t0 + TT])
                ss_sb = ccp.tile([128, TT], F32, tag="ss")
                nc.sync.dma_start(ss_sb[:], ss[:, t0:t0 + TT])

                # lora A: xa[48, TT]
                ps_xa = psa.tile([128, 512], F32, tag="psa")
                for d in range(32):
                    nc.tensor.matmul(ps_xa[0:48, 0:TT], wa_sb[:, d, :],
                                     x_sb[:, d, :], start=(d == 0),
                                     stop=(d == 31))
                xa_sb = xap.tile([48, TT], BF, tag="xa")
                nc.scalar.copy(xa_sb[:], ps_xa[0:48, 0:TT])

                # Q and K, RoPE'd into Q_sb/K_sb
                for dst_sb, w_sb, lora in ((Q_sb, wq_sb, True),
                                           (K_sb, wk_sb, False)):
                    for pair in range(2):
                        ps_pair = []
                        for ri in range(2):
                            f0 = pair * 256 + ri * 128
                            ps = psa.tile([128, 512], F32, tag="psa")
                            for d in range(32):
                                nc.tensor.matmul(
                                    ps[:, 0:TT], w_sb[:, d, f0:f0 + 128],
                                    x_sb[:, d, :], start=(d == 0),
                                    stop=(d == 31 and not lora))
                            if lora:
                                nc.tensor.matmul(
                                    ps[:, 0:TT], wqb_sb[:, f0:f0 + 128],
                                    xa_sb[0:16, :], start=False, stop=True)
                            ps_pair.append(ps)
                        ps_r, ps_i = ps_pair
                        toff = tt * TT
                        t1 = tmpp.tile([128, TT], F32, tag="t")
                        nc.vector.tensor_mul(t1[:], ps_r[:, 0:TT], cc_sb[:])
                        t2 = tmpp.tile([128, TT], F32, tag="t")
                        nc.vector.tensor_mul(t2[:], ps_i[:, 0:TT], ss_sb[:])
                        st_r = stp.tile([128, TT], BF, tag="st")
                        nc.vector.tensor_tensor(
                            st_r[:], t1[:], t2[:], mybir.AluOpType.subtract)
                        t3 = tmpp.tile([128, TT], F32, tag="t")
                        nc.vector.tensor_mul(t3[:], ps_r[:, 0:TT], ss_sb[:])
                        t4 = tmpp.tile([128, TT], F32, tag="t")
                        nc.vector.tensor_mul(t4[:], ps_i[:, 0:TT], cc_sb[:])
                        st_i = stp.tile([128, TT], BF, tag="st")
                        nc.vector.tensor_tensor(
                            st_i[:], t3[:], t4[:], mybir.AluOpType.add)
                        # shuffle into head-contiguous blocks: head h of this
                        # pair = [r half; i half] on partitions [0:64|64:128]
                        for hh in range(2):
                            h_loc = 2 * pair + hh
                            nc.sync.dma_start(
                                dst_sb[0:64, h_loc, toff:toff + TT],
                                st_r[hh * 64:(hh + 1) * 64, :])
                            nc.sync.dma_start(
                                dst_sb[64:128, h_loc, toff:toff + TT],
                                st_i[hh * 64:(hh + 1) * 64, :])

                # V natural: per 128-token block
                for v in range(TT // 128):
                    tb = tt * (TT // 128) + v
                    ps_v = psa.tile([128, 512], F32, tag="psa")
                    for d in range(32):
                        nc.tensor.matmul(
                            ps_v[:], x_sb[:, d, v * 128:(v + 1) * 128],
                            wv_sb[:, d, :], start=(d == 0), stop=False)
                    nc.tensor.matmul(
                        ps_v[:], xa_sb[32:48, v * 128:(v + 1) * 128],
                        wvb_sb[32:48, :], start=False, stop=True)
                    nc.scalar.copy(V_sb[:, tb, :], ps_v[:])

                if tt == 1:
                    attn_half(0)
            attn_half(1)

            # ---- stage C: wo ----
            for nt in range(8):
                wo_sb = wop.tile([128, 4, 512], BF, tag="wo")
                nc.sync.dma_start(
                    wo_sb[:],
                    woT.rearrange("(o p) n -> p o n",
                                  p=128)[:, :, nt * 512:(nt + 1) * 512])
                for tb in range(8):
                    ps_o = psc.tile([128, 512], F32, tag="psa")
                    for k in range(4):
                        nc.tensor.matmul(
                            ps_o[:], OT_sb[:, k, tb * 128:(tb + 1) * 128],
                            wo_sb[:, k, :], start=(k == 0), stop=(k == 3))
                    o_sb = outp.tile([128, 512], F32, tag="o")
                    nc.scalar.copy(o_sb[:], ps_o[:])
                    nc.sync.dma_start(
                        y[b * S + tb * 128:b * S + (tb + 1) * 128,
                          nt * 512:(nt + 1) * 512], o_sb[:])

    nc.compile()
    return nc


_CACHE = {}


def _get_program(causal):
    if causal not in _CACHE:
        _CACHE[causal] = _build_program(causal)
    return _CACHE[causal]


def kernel(x, wq_w, wq_a, wq_b, wk_w, wv_w, wv_a, wv_b, wo_w,
           freqs_cos, freqs_sin, mask, start_pos=0, _trace=False):
    assert int(np.asarray(start_pos)) == 0
    shared, cores, causal = _host_prep(
        x, wq_w, wq_a, wq_b, wk_w, wv_w, wv_a, wv_b, wo_w,
        freqs_cos, freqs_sin, mask)
    nc = _get_program(causal)
    in_maps = []
    for c in range(N_CORES):
        m = dict(xT=shared["xT"], waT=shared["waT"], cc=shared["cc"],
                 ss=shared["ss"], maskp=shared["maskp"])
        m.update(cores[c])
        in_maps.append(m)
    res = run_bass_kernel_spmd(nc, in_maps, list(range(N_CORES)),
                               trace=_trace)
    kernel._last_results = res
    acc = np.zeros((T, D), np.float32)
    for c in range(N_CORES):
        acc += np.asarray(res.results[c]["y"], np.float32)
    out = acc.reshape(B, S, D)
    return out.astype(np.asarray(x).dtype, copy=False)



# revision 5
# speedup vs baseline: 1.0626x; 1.0626x over previous
"""Trainium2 Bass kernel for LoRA attention prefill (B=4, S=1024, D=4096, H=32).

Sharding: tensor-parallel over heads. Each of the 8 cores computes 4 heads
(512 of the 4096 q/k/v features, column-shard of wq/wk/wv + LoRA B) and a
row-shard of wo, producing a full-shape [T, D] bf16 partial output; partials
are summed on the host.

v2 design (vs the v1 feature-stationary kernel):
  - Stage A is token-stationary: per 128-token block, the x-block is the PE
    stationary operand and the packed [wq|wk|wv|wa] weights stream as the
    moving operand in 512-col matmuls.  RoPE is applied on the free axis
    (feature pairs are strided slices), then one XBAR DMA block-transpose
    per token block produces the [HD, tok] layout the scores matmuls need.
    This kills v1's slow 64-partition scatter DMAs that stalled attention.
  - Attention runs in the natural [query, key] layout: softmax denominators
    come free from the exp activation's accum_out (sum over the free axis),
    reciprocals are [128,1] per query block (v1 burned 106us in 1-partition
    reciprocals), probs are normalized before PV, and the PV operand is
    produced by XBAR block-transposes.  The ones-column denominator matmuls
    and fp32 broadcast matmuls of v1 disappear, and causality is exploited
    at 128-key granularity (25% fewer score/PV columns).
  - Output partials are written in bf16 (half the DMA-out bytes).
"""
import sys
from contextlib import ExitStack

sys.path.insert(0, "/opt/trn_rl_repo")

import numpy as np
import ml_dtypes

import concourse.bass as bass
import concourse.mybir as mybir
import concourse.tile as tile
from concourse import bacc
from concourse.bass_utils import run_bass_kernel_spmd
from concourse.tile import TileContext

B, S, D = 4, 1024, 4096
H, HD = 32, 128
R = 16
LORA_SCALE = 2.0
N_CORES = 8
HPC = H // N_CORES            # heads per core = 4
FPC = HPC * HD                # features per core = 512
T = B * S                     # 4096 tokens
NTB = S // 128                # 128-token blocks per batch = 8
WCOLS = 3 * FPC + 48          # packed weight columns: q | k | v | lora-a
SCALE = float(1.0 / np.sqrt(HD))
BF = mybir.dt.bfloat16
F32 = mybir.dt.float32


def _bf(a):
    return np.ascontiguousarray(np.asarray(a, np.float32).astype(ml_dtypes.bfloat16))


def _mask_kind(mask):
    mask = np.asarray(mask, np.float32)
    if not mask.any():
        return "zero"
    iu = np.triu_indices(S, k=1)
    il = np.tril_indices(S, k=0)
    if (mask[iu] <= -1e8).all() and (mask[il] == 0).all():
        d = mask[0:128, 0:128]
        for qb in range(1, NTB):
            if not np.array_equal(mask[qb * 128:(qb + 1) * 128,
                                       qb * 128:(qb + 1) * 128], d):
                return "general"
        return "causal"
    return "general"


def _host_prep(x, wq_w, wq_a, wq_b, wk_w, wv_w, wv_a, wv_b, wo_w,
               freqs_cos, freqs_sin, mask):
    x2 = np.asarray(x, np.float32).reshape(T, D)
    xT = _bf(x2.T)                                   # [D, T]

    kind = _mask_kind(mask)
    mask = np.asarray(mask, np.float32)
    sq = np.float32(np.sqrt(HD))
    if kind == "causal":
        maskd = np.ascontiguousarray(mask[0:128, 0:128] * sq)       # [128,128] f32
    elif kind == "general":
        # natural layout [q, k], pre-scaled, bf16, rearranged [128, qb, S]
        mp = (mask * sq).reshape(NTB, 128, S).transpose(1, 0, 2)
        maskd = _bf(np.ascontiguousarray(mp))
    else:
        maskd = None

    cosE = np.asarray(freqs_cos, np.float32).reshape(NTB, 128, 64)
    sinE = np.asarray(freqs_sin, np.float32).reshape(NTB, 128, 64)
    cosE = np.ascontiguousarray(cosE.transpose(1, 0, 2))  # [128, tb, 64]
    sinE = np.ascontiguousarray(sinE.transpose(1, 0, 2))

    shared = dict(xT=xT, cosE=cosE, sinE=sinE)
    if maskd is not None:
        shared["maskd"] = maskd

    cores = []
    for c in range(N_CORES):
        sl = slice(c * FPC, (c + 1) * FPC)
        w = np.zeros((D, WCOLS), np.float32)
        w[:, 0:FPC] = np.asarray(wq_w, np.float32)[sl, :].T
        w[:, FPC:2 * FPC] = np.asarray(wk_w, np.float32)[sl, :].T
        w[:, 2 * FPC:3 * FPC] = np.asarray(wv_w, np.float32)[sl, :].T
        w[:, 3 * FPC:3 * FPC + 16] = np.asarray(wq_a, np.float32).T
        w[:, 3 * FPC + 32:3 * FPC + 48] = np.asarray(wv_a, np.float32).T
        cores.append(dict(
            wpk=_bf(w),
            wqbT=_bf((np.asarray(wq_b, np.float32)[sl, :] * LORA_SCALE).T),
            wvbT=_bf((np.asarray(wv_b, np.float32)[sl, :] * LORA_SCALE).T),
            woT=_bf(np.asarray(wo_w, np.float32)[:, sl].T),
        ))
    return shared, cores, kind


def _build_program(kind):
    nc = bacc.Bacc("TRN2", num_devices=N_CORES)
    causal = kind == "causal"

    xT = nc.dram_tensor("xT", [D, T], BF, kind="ExternalInput").ap()
    wpk = nc.dram_tensor("wpk", [D, WCOLS], BF, kind="ExternalInput").ap()
    wqbT = nc.dram_tensor("wqbT", [R, FPC], BF, kind="ExternalInput").ap()
    wvbT = nc.dram_tensor("wvbT", [R, FPC], BF, kind="ExternalInput").ap()
    woT = nc.dram_tensor("woT", [FPC, D], BF, kind="ExternalInput").ap()
    cosE = nc.dram_tensor("cosE", [128, NTB, 64], F32, kind="ExternalInput").ap()
    sinE = nc.dram_tensor("sinE", [128, NTB, 64], F32, kind="ExternalInput").ap()
    if kind == "causal":
        maskd = nc.dram_tensor("maskd", [128, 128], F32, kind="ExternalInput").ap()
    elif kind == "general":
        maskd = nc.dram_tensor("maskd", [128, NTB, S], BF, kind="ExternalInput").ap()
    y = nc.dram_tensor("y", [T, D], BF, kind="ExternalOutput").ap()

    # general-mask variant carries a 16KB/partition mask: shrink elsewhere
    xbufs = 2 if kind == "general" else 3
    prbufs = 2 if kind == "general" else 3
    tmpbufs = 4 if kind == "general" else 6
    obufs = 2 if kind == "general" else 4

    with TileContext(nc) as tc, ExitStack() as ctx:
        wpool = ctx.enter_context(tc.tile_pool(name="wpool", bufs=1))
        xpool = ctx.enter_context(tc.tile_pool(name="xpool", bufs=xbufs))
        natp = ctx.enter_context(tc.tile_pool(name="natp", bufs=4))
        tmpp = ctx.enter_context(tc.tile_pool(name="tmpp", bufs=tmpbufs))
        qkp = ctx.enter_context(tc.tile_pool(name="qkp", bufs=1))
        vp = ctx.enter_context(tc.tile_pool(name="vp", bufs=1))
        otp = ctx.enter_context(tc.tile_pool(name="otp", bufs=1))
        xap = ctx.enter_context(tc.tile_pool(name="xap", bufs=2))
        prp = ctx.enter_context(tc.tile_pool(name="prp", bufs=prbufs))
        ptp = ctx.enter_context(tc.tile_pool(name="ptp", bufs=2))
        denp = ctx.enter_context(tc.tile_pool(name="denp", bufs=2))
        wop = ctx.enter_context(tc.tile_pool(name="wop", bufs=2))
        outp = ctx.enter_context(tc.tile_pool(name="outp", bufs=obufs))
        psm = ctx.enter_context(tc.tile_pool(name="psm", bufs=6, space="PSUM"))
        psa = ctx.enter_context(tc.tile_pool(name="psa", bufs=2, space="PSUM"))

        # ---- resident tensors ----
        w_sb = wpool.tile([128, 32, WCOLS], BF, tag="wpk")
        nc.sync.dma_start(w_sb[:], wpk.rearrange("(o p) f -> p o f", p=128))
        wqb_sb = wpool.tile([R, FPC], BF, tag="wqb")
        nc.sync.dma_start(wqb_sb[:], wqbT[:])
        wvb_sb = wpool.tile([48, FPC], BF, tag="wvb")
        nc.sync.dma_start(wvb_sb[32:48, :], wvbT[:])
        cos_sb = wpool.tile([128, NTB, 64], F32, tag="cos")
        nc.sync.dma_start(cos_sb[:], cosE[:])
        sin_sb = wpool.tile([128, NTB, 64], F32, tag="sin")
        nc.sync.dma_start(sin_sb[:], sinE[:])
        if kind == "causal":
            mask_sb = wpool.tile([128, 128], F32, tag="mask")
            nc.sync.dma_start(mask_sb[:], maskd[:])
        elif kind == "general":
            mask_sb = wpool.tile([128, NTB, S], BF, tag="mask")
            nc.sync.dma_start(mask_sb[:], maskd[:])

        def load_x(b, tb):
            x_sb = xpool.tile([128, 32, 128], BF, tag="x")
            t0 = b * S + tb * 128
            nc.sync.dma_start(
                x_sb[:],
                xT.rearrange("(o p) t -> p o t", p=128)[:, :, t0:t0 + 128])
            return x_sb

        def xa_pass(x_sb, xa_ps):
            # feature-stationary lora-A: out [48, 128 tok]
            for d in range(32):
                nc.tensor.matmul(xa_ps[0:48, 0:128], w_sb[:, d, 3 * FPC:],
                                 x_sb[:, d, :], start=(d == 0), stop=(d == 31))

        def rope(ps, nat_sb, tb):
            # ps [128 tok, 512 feat] f32 -> nat_sb [128, 512] bf16, rotated
            pv = ps[:].rearrange("p (h k two) -> p h k two", h=HPC, two=2)
            ne = nat_sb[:].rearrange("p (h k two) -> p h k two", h=HPC, two=2)
            cbc = cos_sb[:, tb, None, :].to_broadcast((128, HPC, 64))
            sbc = sin_sb[:, tb, None, :].to_broadcast((128, HPC, 64))
            q_e = pv[:, :, :, 0]
            q_o = pv[:, :, :, 1]
            t1 = tmpp.tile([128, HPC, 64], F32, tag="t")
            nc.vector.tensor_tensor(t1[:], q_e, cbc, mybir.AluOpType.mult)
            t2 = tmpp.tile([128, HPC, 64], F32, tag="t")
            nc.vector.tensor_tensor(t2[:], q_o, sbc, mybir.AluOpType.mult)
            nc.vector.tensor_tensor(ne[:, :, :, 0], t1[:], t2[:],
                                    mybir.AluOpType.subtract)
            t3 = tmpp.tile([128, HPC, 64], F32, tag="t")
            nc.vector.tensor_tensor(t3[:], q_e, sbc, mybir.AluOpType.mult)
            t4 = tmpp.tile([128, HPC, 64], F32, tag="t")
            nc.vector.tensor_tensor(t4[:], q_o, cbc, mybir.AluOpType.mult)
            nc.vector.tensor_tensor(ne[:, :, :, 1], t3[:], t4[:],
                                    mybir.AluOpType.add)

        for b in range(B):
            QT_sb = qkp.tile([128, HPC, S], BF, tag="QT")
            KT_sb = qkp.tile([128, HPC, S], BF, tag="KT")
            V_sb = vp.tile([128, NTB, FPC], BF, tag="V")
            OT_sb = otp.tile([128, HPC, S], BF, tag="OT")

            # ---------------- stage A ----------------
            x_tiles = [load_x(b, 0), load_x(b, 1)]
            # bootstrap lora-A for tb 0
            xa_ps0 = psa.tile([128, 512], F32, tag="psa")
            xa_pass(x_tiles[0], xa_ps0)
            xaT_prev = xap.tile([48, 128], BF, tag="xaT")
            nc.scalar.copy(xaT_prev[:], xa_ps0[0:48, 0:128])

            for tb in range(NTB):
                x_sb = x_tiles[tb]
                if tb + 2 < NTB:
                    x_tiles.append(load_x(b, tb + 2))
                q_ps = psm.tile([128, 512], F32, tag="psm")
                k_ps = psm.tile([128, 512], F32, tag="psm")
                v_ps = psm.tile([128, 512], F32, tag="psm")
                if tb + 1 < NTB:
                    xa_ps = psa.tile([128, 512], F32, tag="psa")
                for d in range(32):
                    xb = x_sb[:, d, :]
                    nc.tensor.matmul(q_ps[:], xb, w_sb[:, d, 0:FPC],
                                     start=(d == 0), stop=False)
                    nc.tensor.matmul(k_ps[:], xb, w_sb[:, d, FPC:2 * FPC],
                                     start=(d == 0), stop=(d == 31))
                    nc.tensor.matmul(v_ps[:], xb, w_sb[:, d, 2 * FPC:3 * FPC],
                                     start=(d == 0), stop=False)
                    if tb + 1 < NTB:
                        nc.tensor.matmul(
                            xa_ps[0:48, 0:128], w_sb[:, d, 3 * FPC:],
                            x_tiles[tb + 1][:, d, :],
                            start=(d == 0), stop=(d == 31))
                # lora-B tails close the q/v accumulations
                nc.tensor.matmul(q_ps[:], xaT_prev[0:16, :], wqb_sb[:],
                                 start=False, stop=True)
                nc.tensor.matmul(v_ps[:], xaT_prev[32:48, :], wvb_sb[32:48, :],
                                 start=False, stop=True)
                if tb + 1 < NTB:
                    xaT_prev = xap.tile([48, 128], BF, tag="xaT")
                    nc.scalar.copy(xaT_prev[:], xa_ps[0:48, 0:128])

                # RoPE + transpose for Q and K; V copies straight out
                qnat = natp.tile([128, 512], BF, tag="nat")
                rope(q_ps, qnat, tb)
                knat = natp.tile([128, 512], BF, tag="nat")
                rope(k_ps, knat, tb)
                nc.scalar.copy(V_sb[:, tb, :], v_ps[:])
                nc.sync.dma_start_transpose(
                    QT_sb[:, :, tb * 128:(tb + 1) * 128], qnat[:])
                nc.sync.dma_start_transpose(
                    KT_sb[:, :, tb * 128:(tb + 1) * 128], knat[:])

            # ---------------- attention ----------------
            def scores_block(l, qh):
                # probsT tile [128, 8, 512]: [key-in-block, kb, local query]
                pt = ptp.tile([128, NTB, 512], BF, tag="pt")
                den = denp.tile([128, 4, 2], F32, tag="den")
                rec = denp.tile([128, 4], F32, tag="rec")
                for j in range(4):
                    qb = qh * 4 + j
                    q0 = qb * 128
                    kmax = (qb + 1) * 128 if causal else S
                    nk = kmax // 128
                    probs = prp.tile([128, 1024], BF, tag="probs")
                    pieces = []
                    for p0 in range(0, kmax, 512):
                        pw = min(512, kmax - p0)
                        ps_sc = psm.tile([128, 512], F32, tag="psm")
                        nc.tensor.matmul(
                            ps_sc[:, 0:pw], QT_sb[:, l, q0:q0 + 128],
                            KT_sb[:, l, p0:p0 + pw], start=True, stop=True)
                        pieces.append((p0, pw, ps_sc))
                    if causal:
                        # triangular mask on the diagonal 128-key block
                        p0, pw, ps_sc = pieces[-1]
                        off = qb * 128 - p0
                        nc.vector.tensor_add(
                            ps_sc[:, off:off + 128], ps_sc[:, off:off + 128],
                            mask_sb[:])
                    elif kind == "general":
                        for p0, pw, ps_sc in pieces:
                            nc.vector.tensor_add(
                                ps_sc[:, 0:pw], ps_sc[:, 0:pw],
                                mask_sb[:, qb, p0:p0 + pw])
                    for pi, (p0, pw, ps_sc) in enumerate(pieces):
                        nc.scalar.activation(
                            probs[:, p0:p0 + pw], ps_sc[:, 0:pw],
                            mybir.ActivationFunctionType.Exp, scale=SCALE,
                            accum_out=den[:, j, pi:pi + 1])
                    if len(pieces) > 1:
                        nc.vector.tensor_add(den[:, j, 0:1], den[:, j, 0:1],
                                             den[:, j, 1:2])
                    nc.vector.reciprocal(rec[:, j:j + 1], den[:, j, 0:1])
                    nc.vector.tensor_scalar_mul(
                        probs[:, 0:kmax], probs[:, 0:kmax], rec[:, j:j + 1])
                    nc.sync.dma_start_transpose(
                        pt[:, 0:nk, j * 128:(j + 1) * 128], probs[:, 0:kmax])
                return pt

            def pv_block(l, qh, pt):
                ps_ot = psm.tile([128, 512], F32, tag="psm")
                nfull = qh * 4 if causal else NTB
                for kb in range(nfull):
                    nc.tensor.matmul(
                        ps_ot[:], V_sb[:, kb, l * 128:(l + 1) * 128],
                        pt[:, kb, :], start=(kb == 0),
                        stop=(not causal and kb == NTB - 1))
                if causal:
                    # diagonal 512x512 zone: per (qb, kb) valid 128-col pieces
                    for j in range(4):
                        qb = qh * 4 + j
                        for kb in range(nfull, qb + 1):
                            nc.tensor.matmul(
                                ps_ot[:, j * 128:(j + 1) * 128],
                                V_sb[:, kb, l * 128:(l + 1) * 128],
                                pt[:, kb, j * 128:(j + 1) * 128],
                                start=(nfull == 0 and kb == 0),
                                stop=(kb == qb))
                nc.scalar.copy(OT_sb[:, l, qh * 512:(qh + 1) * 512], ps_ot[:])

            # software-pipelined issue: PV lags scores by one stage
            stages = [(l, qh) for l in range(HPC) for qh in range(2)]
            pts = {}
            for si, (l, qh) in enumerate(stages):
                pts[(l, qh)] = scores_block(l, qh)
                if si >= 1:
                    pl, pq = stages[si - 1]
                    pv_block(pl, pq, pts.pop((pl, pq)))
            pl, pq = stages[-1]
            pv_block(pl, pq, pts.pop((pl, pq)))

            # ---------------- stage C ----------------
            for nt in range(8):
                wo_sb = wop.tile([128, HPC, 512], BF, tag="wo")
                nc.sync.dma_start(
                    wo_sb[:],
                    woT.rearrange("(o p) n -> p o n",
                                  p=128)[:, :, nt * 512:(nt + 1) * 512])
                for tb in range(8):
                    ps_o = psm.tile([128, 512], F32, tag="psm")
                    for k in range(HPC):
                        nc.tensor.matmul(
                            ps_o[:], OT_sb[:, k, tb * 128:(tb + 1) * 128],
                            wo_sb[:, k, :], start=(k == 0), stop=(k == 3))
                    o_sb = outp.tile([128, 512], BF, tag="o")
                    if tb % 2 == 0:
                        nc.scalar.copy(o_sb[:], ps_o[:])
                    else:
                        nc.vector.tensor_copy(o_sb[:], ps_o[:])
                    nc.sync.dma_start(
                        y[b * S + tb * 128:b * S + (tb + 1) * 128,
                          nt * 512:(nt + 1) * 512], o_sb[:])

    nc.compile()
    return nc


_CACHE = {}


def _get_program(kind):
    if kind not in _CACHE:
        _CACHE[kind] = _build_program(kind)
    return _CACHE[kind]


def kernel(x, wq_w, wq_a, wq_b, wk_w, wv_w, wv_a, wv_b, wo_w,
           freqs_cos, freqs_sin, mask, start_pos=0, _trace=False):
    assert int(np.asarray(start_pos)) == 0
    shared, cores, kind = _host_prep(
        x, wq_w, wq_a, wq_b, wk_w, wv_w, wv_a, wv_b, wo_w,
        freqs_cos, freqs_sin, mask)
    nc = _get_program(kind)
    in_maps = []
    for c in range(N_CORES):
        m = dict(shared)
        m.update(cores[c])
        in_maps.append(m)
    res = run_bass_kernel_spmd(nc, in_maps, list(range(N_CORES)),
                               trace=_trace)
    kernel._last_results = res
    acc = np.zeros((T, D), np.float32)
    for c in range(N_CORES):
        acc += np.asarray(res.results[c]["y"], np.float32)
    out = acc.reshape(B, S, D)
    return out.astype(np.asarray(x).dtype, copy=False)
